# revision 45
# baseline (speedup 1.0000x reference)
"""CRF attention layer (nn_CRFAttentionLayer) for 8 TRN2 NeuronCores.

Math (K=2 iterations, N=8192, D=256):
    H_proj = H @ W.T + b
    S      = H_proj @ H_proj.T          (masked where sim_mat == 0)
    lamb   = softmax(S, axis=1)
    H      = (ALPHA*Q + BETA*(lamb @ H)) / (ALPHA + BETA*sum(lamb))

Sharding: rows split across 8 cores (1024 local rows each).  Scores run as
fp8 DoubleRow matmuls in S^T layout (keys on partitions, 512-row passes).
It0 projects all keys redundantly from host-provided Q^T (no collective on
the critical path); between iterations one AllGather moves the updated
H1 (fp8) + its projection (fp8) to every core.

Softmax trick: the diagonal S_rr = ||Hp_r||^2 dominates every row, and the
per-row normalization o/z cancels any per-row-consistent shift, so the
kernel accumulates S - ||Hp_r||^2 - 1 directly in PSUM: the additive mask
((sim-1)*192, fp8-exact) is applied by an identity DoubleRow matmul, and
the per-row bias rides in a spare DoubleRow slot as two fp8 terms
(coarse + residual, +-0.5 quantization cancels row-consistently).  The
scalar engine then exps PSUM straight to fp8 values — no vector-engine
work in the inner loop — and the value/row-sum matmuls run fp8 DoubleRow.
End-to-end rel err vs f64: ~1e-3 (tol 2e-2).
"""

import sys

sys.path.insert(0, "/opt/trn_rl_repo")

import numpy as np
import ml_dtypes

import concourse.bass as bass
import concourse.tile as tile
from concourse import bacc, mybir
from concourse.bass_utils import run_bass_kernel_spmd



FP = mybir.dt.float32
BF = mybir.dt.bfloat16
F8 = mybir.dt.float8e4
AF = mybir.ActivationFunctionType
AX = mybir.AxisListType
OP = mybir.AluOpType
DR = mybir.MatmulPerfMode.DoubleRow

N, D = 8192, 256
NC = 8
LR = N // NC          # 1024 local rows per core
RW = 512              # rows per attention pass
NPASS = LR // RW      # 2
NPAIR = N // 256      # 32 pair-groups of 256 keys
ALPHA, BETA = 50.0, 1.0
MSHIFT = 1.0          # safety margin in the per-row shift
MBIG = 192.0          # additive mask magnitude (fp8-exact)
K_ITERS = 2

F8NP = ml_dtypes.float8_e4m3
BFNP = ml_dtypes.bfloat16


def _t(pool, shape, dtype, tag, bufs=None):
    return pool.tile(list(shape), dtype, tag=tag, name=tag, bufs=bufs)


def build():
    nc = bacc.Bacc("TRN2", target_bir_lowering=False, debug=False, num_devices=NC)

    sim_t = nc.declare_dram_parameter("sim_t", [N, LR], F8, isOutput=False)
    qt_full = nc.declare_dram_parameter("qt_full", [D, N], BF, isOutput=False)
    q_t = nc.declare_dram_parameter("q_t", [D, LR], BF, isOutput=False)
    q8_in = nc.declare_dram_parameter("q8", [N, D], F8, isOutput=False)
    q_loc = nc.declare_dram_parameter("q_loc", [LR, D], FP, isOutput=False)
    w_in = nc.declare_dram_parameter("w", [D, D], FP, isOutput=False)
    b_in = nc.declare_dram_parameter("b", [D, 1], FP, isOutput=False)
    out = nc.declare_dram_parameter("out", [LR, D], FP, isOutput=True)

    id_bf_d = nc.inline_tensor(np.eye(128, dtype=BFNP), name="id_bf")
    id_f_d = nc.inline_tensor(np.eye(128, dtype=np.float32), name="id_f")
    ones_col_d = nc.inline_tensor(np.ones((128, 1), dtype=BFNP), name="ones_col")
    ones2_d = nc.inline_tensor(np.ones((128, 32), dtype=F8NP), name="ones2")
    # row-bias stationary: row 0 ones in both sub-rows -> out[f,r] += M2[0,0,r]+M2[0,1,r]
    _t2 = np.zeros((128, 256), dtype=F8NP)
    _t2[0, :] = 1.0
    t2_d = nc.inline_tensor(_t2, name="t2c")

    with tile.TileContext(nc) as tc:
        with (
            tc.tile_pool(name="pers", bufs=1) as pers,
            tc.tile_pool(name="simt", bufs=1) as simt_pool,
            tc.tile_pool(name="dram", bufs=1, space="DRAM") as dram,
        ):
            # ---- constants ----
            id_bf = _t(pers, (128, 128), BF, "id_bf")
            nc.sync.dma_start(id_bf[:], id_bf_d.ap())
            id_f = _t(pers, (128, 128), FP, "id_f")
            nc.sync.dma_start(id_f[:], id_f_d.ap())
            ones_col = _t(pers, (128, 1), BF, "ones_col")
            nc.sync.dma_start(ones_col[:], ones_col_d.ap())
            ones2 = _t(pers, (128, 32), F8, "ones2")
            nc.sync.dma_start(ones2[:], ones2_d.ap())
            ones2_3 = ones2.rearrange("p (i f) -> p i f", f=16)
            t2c = _t(pers, (128, 256), F8, "t2c")
            nc.sync.dma_start(t2c[:], t2_d.ap())
            t2c_3 = t2c.rearrange("p (i f) -> p i f", i=2)
            # warm the scalar engine's exp table before the first real exp
            etw = _t(pers, (1, 1), FP, "etw")
            nc.scalar.activation(etw[:], id_f[0:1, 0:1], AF.Exp)
            bvec = []
            for kh in range(2):
                bt = _t(pers, (128, 1), FP, f"bvec{kh}")
                nc.sync.dma_start(bt[:], b_in[128 * kh : 128 * (kh + 1), 0:1])
                bvec.append(bt)

            # ---- W^T in bf16: wt[kh][k=128, d=256] = W[d, kh*128+k] ----
            wt = [_t(pers, (128, 256), BF, f"wt{kh}") for kh in range(2)]
            with (
                tc.tile_pool(name="wsb", bufs=2) as wsb,
                tc.tile_pool(name="wps", bufs=2, space="PSUM") as wps,
            ):
                for dh in range(2):
                    wl = _t(wsb, (128, 256), FP, "wl")
                    nc.sync.dma_start(wl[:], w_in[128 * dh : 128 * (dh + 1), :])
                    wc = _t(wsb, (128, 256), BF, "wc")
                    nc.vector.tensor_copy(wc[:], wl[:])
                    for kh in range(2):
                        wp = _t(wps, (128, 128), BF, "wp")
                        nc.tensor.transpose(wp[:], wc[:, 128 * kh : 128 * (kh + 1)], id_bf[:])
                        nc.vector.tensor_copy(wt[kh][:, 128 * dh : 128 * (dh + 1)], wp[:])

            # ---- persistent state ----
            # hp8b[g8][p, dh*LR + n] = H_projT[dh*128+p, g8*LR + n]  (fp8, per block
            # so it0 attention can start before the whole projection lands)
            hp8b = [_t(pers, (128, 2 * LR), F8, f"hp8b{g8}") for g8 in range(NC)]
            hp8b_3 = [t.rearrange("p (i n) -> p i n", i=2) for t in hp8b]
            # hp8_l: same layout, local rows only
            hp8_l = _t(pers, (128, 2 * LR), F8, "hp8_l")
            hp8_l3 = hp8_l.rearrange("p (i n) -> p i n", i=2)
            # qtfb[g8][kh][k, n] = Q[g8*LR + n, kh*128+k]  (bf16; it0 full proj,
            # split per key block so projection starts as soon as block 0 lands)
            qtfb = [
                [_t(pers, (128, LR), BF, f"qtf{g8}_{kh}") for kh in range(2)]
                for g8 in range(NC)
            ]
            # hT[kh][k, n] = H[n, kh*128+k]  (bf16, local rows; local proj moving)
            hT = [_t(pers, (128, LR), BF, f"hT{kh}") for kh in range(2)]
            # hpk[g]: values stationary, group g = keys [256g, 256g+256):
            #   hpk[g][p, i*256+f] = H[256g + 128i + p, f]  (fp8)
            hpk = [_t(pers, (128, 512), F8, f"hpk{g}") for g in range(NPAIR)]
            # resident local Q (f32) for the epilogue
            ql = [_t(pers, (128, D), FP, f"ql{t}") for t in range(LR // 128)]
            # additive mask (0 / -192) in S^T layout, fp8: pair tile g holds
            # chunks (2g, 2g+1): simt[g][p, i*LR + r] = mask[r_local, 256g+128i+p]
            simt = [_t(simt_pool, (128, 2 * LR), F8, f"simT{g}") for g in range(NPAIR)]
            # row-bias moving tile: partition 0 carries (coarse, residual) fp8
            # row terms; other partitions zero (stationary t2c zeroes them).
            m2 = _t(pers, (128, 2 * LR), F8, "m2")
            nc.vector.memset(m2[:], 0.0)
            m2_3 = m2.rearrange("p (i r) -> p i r", i=2)

            ot_sb = _t(pers, (128, 2 * LR), FP, "ot")
            zsb = _t(pers, (1, LR), FP, "zsb")

            # ---- collective buffers (one gather: H1 fp8 + Hp1 fp8) ----
            cc1_in = dram.tile([256, 2048], F8)
            cc1_out = dram.tile([2048, 2048], F8, addr_space="Shared")
            cc1_in_h = cc1_in.rearrange("a (b c) -> (a b) c", b=8)    # [2048, 256]
            cc1_out_h = cc1_out.rearrange("a (b c) -> (a b) c", b=8)  # [16384, 256]
            # tiny warm-up gather: absorbs the ~11us first-collective ncfw
            # latency during it0 (no data deps; runs on the CC stream)
            ccw_in = dram.tile([1, 128], BF)
            ccw_out = dram.tile([8, 128], BF, addr_space="Shared")

            # ---- upfront loads (critical-path first: proj moving, then the
            # tiles the first attention groups touch, then the long tail) ----
            for kh in range(2):
                nc.sync.dma_start(hT[kh][:], q_t[128 * kh : 128 * (kh + 1), :])
            for g8 in range(NC):
                for kh in range(2):
                    nc.sync.dma_start(
                        qtfb[g8][kh][:],
                        qt_full[128 * kh : 128 * (kh + 1), LR * g8 : LR * (g8 + 1)],
                    )
            for g in range(NPAIR):
                for i in range(2):
                    c = 2 * g + i
                    nc.sync.dma_start(
                        hpk[g][:, 256 * i : 256 * (i + 1)],
                        q8_in[256 * g + 128 * i : 256 * g + 128 * (i + 1), :],
                    )
                    nc.sync.dma_start(
                        simt[g][:, LR * i : LR * (i + 1)],
                        sim_t[128 * c : 128 * (c + 1), :],
                    )
            for t in range(LR // 128):
                nc.sync.dma_start(ql[t][:], q_loc[128 * t : 128 * (t + 1), :])

            # =================================================================
            def projection(dest_fn, n_cols, moving, on_dve=False):
                """fp8(W @ H^T + b) from bf16 moving tiles; dest_fn(dh, blk) -> AP.
                on_dve routes the bias-add/cast through the vector engine (keeps
                the scalar engine free for the attention exps)."""
                with tc.tile_pool(name="pj_mm", bufs=2, space="PSUM") as pj_mm:
                    for blk in range(n_cols // 512):
                        for dh in range(2):
                            mm = _t(pj_mm, (128, 512), FP, "hp")
                            nc.tensor.matmul(
                                mm[:], wt[0][:, 128 * dh : 128 * (dh + 1)],
                                moving[0][:, 512 * blk : 512 * (blk + 1)],
                                start=True, stop=False,
                            )
                            nc.tensor.matmul(
                                mm[:], wt[1][:, 128 * dh : 128 * (dh + 1)],
                                moving[1][:, 512 * blk : 512 * (blk + 1)],
                                start=False, stop=True,
                            )
                            if on_dve:
                                nc.vector.tensor_scalar_add(
                                    dest_fn(dh, blk), mm[:], bvec[dh][:, 0:1]
                                )
                            else:
                                nc.scalar.activation(
                                    dest_fn(dh, blk), mm[:], AF.Identity,
                                    bias=bvec[dh][:, 0:1],
                                )

            # =================================================================
            def row_bias(it):
                """m2 partition-0 <- fp8 two-term split of -(||Hp_r||^2 + MSHIFT)."""
                with (
                    tc.tile_pool(name="nm_sb", bufs=2) as nm_sb,
                    tc.tile_pool(name="nm_ps", bufs=2, space="PSUM") as nm_ps,
                ):
                    sq = _t(nm_sb, (128, 2 * LR), BF, "sq")
                    nc.vector.tensor_mul(sq[:], hp8_l[:], hp8_l[:])
                    n2_sb = _t(nm_sb, (1, LR), FP, "n2_sb")
                    for rh in range(2):
                        n2_ps = _t(nm_ps, (1, 512), FP, "n2_ps")
                        for dh in range(2):
                            nc.tensor.matmul(
                                n2_ps[:], ones_col[:],
                                sq[:, dh * LR + 512 * rh : dh * LR + 512 * (rh + 1)],
                                start=(dh == 0), stop=(dh == 1),
                            )
                        nc.vector.tensor_copy(
                            n2_sb[0:1, 512 * rh : 512 * (rh + 1)], n2_ps[:]
                        )
                    nmm = _t(nm_sb, (1, LR), FP, "nmm")
                    nc.vector.tensor_scalar(
                        nmm[:], n2_sb[:], -1.0, -MSHIFT, op0=OP.mult, op1=OP.add
                    )
                    nc.vector.tensor_copy(m2[0:1, 0:LR], nmm[:])     # coarse fp8
                    t1f = _t(nm_sb, (1, LR), FP, "t1f")
                    nc.vector.tensor_copy(t1f[:], m2[0:1, 0:LR])
                    res = _t(nm_sb, (1, LR), FP, "res")
                    nc.vector.tensor_sub(res[:], nmm[:], t1f[:])
                    nc.vector.tensor_copy(m2[0:1, LR : 2 * LR], res[:])  # residual fp8

            # =================================================================
            def attention(it):
                """PSUM: S - mask - rowbias (all fp8 DR) -> ACT exp -> fp8 values."""
                with (
                    tc.tile_pool(name="at_sb", bufs=3) as at_sb,
                    tc.tile_pool(name="at_sc", bufs=2, space="PSUM") as at_sc,
                    tc.tile_pool(name="at_o", bufs=1, space="PSUM") as at_o,
                    tc.tile_pool(name="at_z", bufs=1, space="PSUM") as at_z,
                ):
                    for p in range(NPASS):
                        o_ps = [_t(at_o, (128, RW), FP, f"o{dh}") for dh in range(2)]
                        z_ps = _t(at_z, (16, RW), FP, "z")
                        for g in range(NPAIR):
                            sc = _t(at_sc, (128, 1024), FP, "sc")
                            sgm = simt[g].rearrange("p (i r) -> p i r", i=2)
                            # scores for both halves first, then both row-bias
                            # adds: the t2c stationary loads land back-to-back
                            for i in range(2):
                                c = 2 * g + i
                                nc.tensor.matmul(
                                    sc[:, RW * i : RW * (i + 1)],
                                    hp8b_3[c // 8][:, :, 128 * (c % 8) : 128 * (c % 8 + 1)],
                                    hp8_l3[:, :, RW * p : RW * (p + 1)],
                                    start=True, stop=False, perf_mode=DR,
                                )
                            for i in range(2):
                                nc.tensor.matmul(
                                    sc[:, RW * i : RW * (i + 1)],
                                    t2c_3[:, :, :],
                                    m2_3[:, :, RW * p : RW * (p + 1)],
                                    start=False, stop=True, perf_mode=DR,
                                )
                            vexp = _t(at_sb, (128, 1024), BF, "vexp")
                            nc.scalar.activation(vexp[:], sc[:], AF.Exp)
                            vexp3 = vexp.rearrange("p (i r) -> p i r", i=2)
                            v2 = _t(at_sb, (128, 1024), F8, "v2")
                            v2m = v2.rearrange("p (i r) -> p i r", i=2)
                            nc.vector.tensor_mul(
                                v2m[:, :, :], vexp3[:, :, :],
                                sgm[:, :, RW * p : RW * (p + 1)],
                            )
                            first, last = (g == 0), (g == NPAIR - 1)
                            hpk3 = hpk[g].rearrange("p (i f) -> p i f", i=2)
                            for dh in range(2):
                                nc.tensor.matmul(
                                    o_ps[dh][:],
                                    hpk3[:, :, 128 * dh : 128 * (dh + 1)],
                                    v2m[:, :, :],
                                    start=first, stop=last, perf_mode=DR,
                                )
                            nc.tensor.matmul(
                                z_ps[:], ones2_3[:, :, :], v2m[:, :, :],
                                start=first, stop=last, perf_mode=DR,
                            )
                        for dh in range(2):
                            nc.scalar.activation(
                                ot_sb[:, 1024 * dh + RW * p : 1024 * dh + RW * (p + 1)],
                                o_ps[dh][:], AF.Copy,
                            )
                        nc.scalar.activation(
                            zsb[0:1, RW * p : RW * (p + 1)], z_ps[0:1, :], AF.Copy
                        )

            # =================================================================
            def epilogue(it, invz):
                with (
                    tc.tile_pool(name="ep_sb", bufs=3) as ep_sb,
                    tc.tile_pool(name="ep_ps", bufs=2, space="PSUM") as ep_ps,
                    tc.tile_pool(name="ep_tp", bufs=2, space="PSUM") as ep_tp,
                ):
                    zp_ps = _t(ep_tp, (128, LR // 128), FP, "zp_ps", bufs=1)
                    for t in range(LR // 128):
                        nc.tensor.transpose(
                            zp_ps[:, t : t + 1], zsb[0:1, 128 * t : 128 * (t + 1)],
                            id_f[0:1, 0:1],
                        )
                    z51 = _t(ep_sb, (128, LR // 128), FP, "z51", bufs=1)
                    nc.vector.tensor_scalar_mul(z51[:], zp_ps[:], ALPHA + BETA)
                    nc.vector.reciprocal(invz[:], z51[:])
                    for t in range(LR // 128):
                        on_ps = _t(ep_ps, (128, D), FP, "on")
                        p, sub = t // (RW // 128), t % (RW // 128)
                        for dh in range(2):
                            nc.tensor.transpose(
                                on_ps[:, 128 * dh : 128 * (dh + 1)],
                                ot_sb[:, 1024 * dh + RW * p + 128 * sub :
                                      1024 * dh + RW * p + 128 * (sub + 1)],
                                id_f[:],
                            )
                        t1 = _t(ep_sb, (128, D), FP, "t1")
                        nc.scalar.activation(
                            t1[:], on_ps[:], AF.Copy, scale=invz[:, t : t + 1]
                        )
                        qs = _t(ep_sb, (128, D), FP, "qs")
                        nc.vector.tensor_scalar_mul(
                            qs[:], ql[t][:], ALPHA / (ALPHA + BETA)
                        )
                        hnew = _t(ep_sb, (128, D), FP, "hnew")
                        nc.vector.tensor_add(hnew[:], t1[:], qs[:])
                        if it == 0:
                            # fp8 H1 for it1 values -> gather buffer
                            h8 = _t(ep_sb, (128, D), F8, "h8")
                            nc.vector.tensor_copy(h8[:], hnew[:])
                            nc.sync.dma_start(
                                cc1_in_h[1024 + 128 * t : 1024 + 128 * (t + 1), :], h8[:]
                            )
                            # bf16 H1^T for it1 projection
                            hb = _t(ep_sb, (128, D), BF, "hb")
                            nc.vector.tensor_copy(hb[:], hnew[:])
                            for kh in range(2):
                                tp = _t(ep_tp, (128, 128), BF, "tp")
                                nc.tensor.transpose(
                                    tp[:], hb[:, 128 * kh : 128 * (kh + 1)], id_bf[:]
                                )
                                nc.vector.tensor_copy(
                                    hT[kh][:, 128 * t : 128 * (t + 1)], tp[:]
                                )
                        else:
                            nc.sync.dma_start(
                                out[128 * t : 128 * (t + 1), :], hnew[:]
                            )

            # =================================================================
            for it in range(K_ITERS):
                invz = _t(pers, (128, LR // 128), FP, f"invz{it}")
                # local rows first (every score matmul needs hp8_l moving) so
                # the row-bias chain starts immediately
                projection(
                    lambda dh, blk: hp8_l[:, dh * LR + 512 * blk : dh * LR + 512 * (blk + 1)],
                    LR, hT,
                )
                row_bias(it)
                if it == 0:
                    # all key blocks, redundant per core, in consume order
                    for g8 in range(NC):
                        projection(
                            lambda dh, blk, g8=g8: hp8b[g8][
                                :, dh * LR + 512 * blk : dh * LR + 512 * (blk + 1)
                            ],
                            LR, qtfb[g8], on_dve=True,
                        )
                if it == 0:
                    # fire the warm-up gather during it0's attention
                    nc.sync.dma_start(ccw_in[:], id_bf_d.ap()[0:1, :])
                    nc.gpsimd.collective_compute(
                        "AllGather",
                        OP.bypass,
                        replica_groups=[list(range(NC))],
                        ins=[ccw_in.opt()],
                        outs=[ccw_out.opt()],
                    )
                else:
                    nc.sync.dma_start(cc1_in[0:128, :], hp8_l[:])
                    # bf16-bitcast views: the CCE slices by ELEMENT count, so
                    # moving the same bytes as bf16 halves the collective time
                    nc.gpsimd.collective_compute(
                        "AllGather",
                        OP.bypass,
                        replica_groups=[list(range(NC))],
                        ins=[cc1_in.bitcast(BF).opt()],
                        outs=[cc1_out.bitcast(BF).opt()],
                    )
                    # gathered projections -> hp8 block tiles
                    for g8 in range(NC):
                        for dh in range(2):
                            nc.sync.dma_start(
                                hp8b[g8][:, dh * LR : dh * LR + LR],
                                cc1_out[256 * g8 : 256 * g8 + 128,
                                        1024 * dh : 1024 * (dh + 1)],
                            )
                    # gathered H1 (fp8, flat rows) -> hpk group tiles
                    for g in range(NPAIR):
                        for i in range(2):
                            r0 = 256 * g + 128 * i
                            blk = r0 // LR
                            lr = r0 - LR * blk
                            nc.sync.dma_start(
                                hpk[g][:, 256 * i : 256 * (i + 1)],
                                cc1_out_h[2048 * blk + 1024 + lr :
                                          2048 * blk + 1024 + lr + 128, :],
                            )
                attention(it)
                epilogue(it, invz)
    nc.compile()
    return nc


def _install_ntff_hook():
    """The agent image's antenv lacks axon_hooks; synthesize it and register
    the ctypes NTFF profile hook so run_bass_kernel_spmd(trace=True) works."""
    import types

    if "antenv.axon_hooks" in sys.modules:
        return
    import antenv
    from trn_agent_boot.trn_boot import _ntff_profile_via_ctypes

    mod = types.ModuleType("antenv.axon_hooks")
    _state = {}
    mod.set_axon_ntff_profile_hook = lambda h: _state.__setitem__("h", h)
    mod.get_axon_ntff_profile_hook = lambda: _state.get("h")
    sys.modules["antenv.axon_hooks"] = mod
    antenv.axon_hooks = mod
    mod.set_axon_ntff_profile_hook(
        _ntff_profile_via_ctypes("/opt/axon/libaxon_pjrt.so")
    )


_NC_CACHE = None


def _get_nc():
    global _NC_CACHE
    if _NC_CACHE is None:
        _NC_CACHE = build()
    return _NC_CACHE


def kernel(Q, sim_mat, W, b, _trace=False, _trace_kwargs=None):
    Q = np.ascontiguousarray(np.asarray(Q, dtype=np.float32))
    sim_mat = np.ascontiguousarray(np.asarray(sim_mat, dtype=np.float32))
    W = np.ascontiguousarray(np.asarray(W, dtype=np.float32))
    b = np.ascontiguousarray(np.asarray(b, dtype=np.float32)).reshape(D, 1)

    s8m = sim_mat.astype(F8NP)                    # 0/1 multiplicative mask
    q8 = np.ascontiguousarray(Q.astype(F8NP))
    qtf = np.ascontiguousarray(Q.T.astype(BFNP))

    in_maps = []
    for g in range(NC):
        sl = slice(g * LR, (g + 1) * LR)
        in_maps.append(
            {
                "sim_t": np.ascontiguousarray(s8m[sl].T),
                "qt_full": qtf,
                "q_t": np.ascontiguousarray(Q[sl].T.astype(BFNP)),
                "q8": q8,
                "q_loc": np.ascontiguousarray(Q[sl]),
                "w": W,
                "b": b,
            }
        )
    nc = _get_nc()
    kw = {}
    if _trace:
        _install_ntff_hook()
        kw["trace"] = True
        kw.update(_trace_kwargs or {})
    res = run_bass_kernel_spmd(nc, in_maps, core_ids=list(range(NC)), **kw)
    outp = np.concatenate(
        [np.asarray(res.results[g]["out"]).reshape(LR, D) for g in range(NC)], axis=0
    ).astype(np.float32)
    if _trace:
        return outp, res
    return outp


if __name__ == "__main__":
    nc = build()
    print("build+compile OK")


# revision 47
# speedup vs baseline: 1.0344x; 1.0344x over previous
"""CRF attention layer (nn_CRFAttentionLayer) for 8 TRN2 NeuronCores.

Math (K=2 iterations, N=8192, D=256):
    H_proj = H @ W.T + b
    S      = H_proj @ H_proj.T          (masked where sim_mat == 0)
    lamb   = softmax(S, axis=1)
    H      = (ALPHA*Q + BETA*(lamb @ H)) / (ALPHA + BETA*sum(lamb))

Sharding: rows split across 8 cores (1024 local rows each).  Scores run as
fp8 DoubleRow matmuls in S^T layout (keys on partitions, 512-row passes).
It0 projects all keys redundantly from host-provided Q^T (no collective on
the critical path); between iterations one AllGather moves the updated
H1 (fp8) + its projection (fp8) to every core.

Softmax trick: the diagonal S_rr = ||Hp_r||^2 dominates every row, and the
per-row normalization o/z cancels any per-row-consistent shift, so the
kernel accumulates S - ||Hp_r||^2 - 1 directly in PSUM: the additive mask
((sim-1)*192, fp8-exact) is applied by an identity DoubleRow matmul, and
the per-row bias rides in a spare DoubleRow slot as two fp8 terms
(coarse + residual, +-0.5 quantization cancels row-consistently).  The
scalar engine then exps PSUM straight to fp8 values — no vector-engine
work in the inner loop — and the value/row-sum matmuls run fp8 DoubleRow.
End-to-end rel err vs f64: ~1e-3 (tol 2e-2).
"""

import sys

sys.path.insert(0, "/opt/trn_rl_repo")

import numpy as np
import ml_dtypes

import concourse.bass as bass
import concourse.tile as tile
from concourse import bacc, mybir
from concourse.bass_utils import run_bass_kernel_spmd



FP = mybir.dt.float32
BF = mybir.dt.bfloat16
F8 = mybir.dt.float8e4
AF = mybir.ActivationFunctionType
AX = mybir.AxisListType
OP = mybir.AluOpType
DR = mybir.MatmulPerfMode.DoubleRow

N, D = 8192, 256
NC = 8
LR = N // NC          # 1024 local rows per core
RW = 512              # rows per attention pass
NPASS = LR // RW      # 2
NPAIR = N // 256      # 32 pair-groups of 256 keys
ALPHA, BETA = 50.0, 1.0
MSHIFT = 1.0          # safety margin in the per-row shift
MBIG = 192.0          # additive mask magnitude (fp8-exact)
K_ITERS = 2

F8NP = ml_dtypes.float8_e4m3
BFNP = ml_dtypes.bfloat16


def _t(pool, shape, dtype, tag, bufs=None):
    return pool.tile(list(shape), dtype, tag=tag, name=tag, bufs=bufs)


def build():
    nc = bacc.Bacc("TRN2", target_bir_lowering=False, debug=False, num_devices=NC)

    sim_t = nc.declare_dram_parameter("sim_t", [N, LR], F8, isOutput=False)
    qt_full = nc.declare_dram_parameter("qt_full", [D, N], BF, isOutput=False)
    q_t = nc.declare_dram_parameter("q_t", [D, LR], BF, isOutput=False)
    q8_in = nc.declare_dram_parameter("q8", [N, D], F8, isOutput=False)
    q_loc = nc.declare_dram_parameter("q_loc", [LR, D], FP, isOutput=False)
    w_in = nc.declare_dram_parameter("w", [D, D], FP, isOutput=False)
    b_in = nc.declare_dram_parameter("b", [D, 1], FP, isOutput=False)
    out = nc.declare_dram_parameter("out", [LR, D], FP, isOutput=True)

    id_bf_d = nc.inline_tensor(np.eye(128, dtype=BFNP), name="id_bf")
    id_f_d = nc.inline_tensor(np.eye(128, dtype=np.float32), name="id_f")
    ones_col_d = nc.inline_tensor(np.ones((128, 1), dtype=BFNP), name="ones_col")
    ones2_d = nc.inline_tensor(np.ones((128, 32), dtype=F8NP), name="ones2")
    # row-bias stationary: row 0 ones in both sub-rows -> out[f,r] += M2[0,0,r]+M2[0,1,r]
    _t2 = np.zeros((128, 256), dtype=F8NP)
    _t2[0, :] = 1.0
    t2_d = nc.inline_tensor(_t2, name="t2c")

    with tile.TileContext(nc) as tc:
        with (
            tc.tile_pool(name="pers", bufs=1) as pers,
            tc.tile_pool(name="simt", bufs=1) as simt_pool,
            tc.tile_pool(name="dram", bufs=1, space="DRAM") as dram,
        ):
            # ---- constants ----
            id_bf = _t(pers, (128, 128), BF, "id_bf")
            nc.sync.dma_start(id_bf[:], id_bf_d.ap())
            id_f = _t(pers, (128, 128), FP, "id_f")
            nc.sync.dma_start(id_f[:], id_f_d.ap())
            ones_col = _t(pers, (128, 1), BF, "ones_col")
            nc.sync.dma_start(ones_col[:], ones_col_d.ap())
            ones2 = _t(pers, (128, 32), F8, "ones2")
            nc.sync.dma_start(ones2[:], ones2_d.ap())
            ones2_3 = ones2.rearrange("p (i f) -> p i f", f=16)
            t2c = _t(pers, (128, 256), F8, "t2c")
            nc.sync.dma_start(t2c[:], t2_d.ap())
            t2c_3 = t2c.rearrange("p (i f) -> p i f", i=2)
            # warm the scalar engine's exp table before the first real exp
            etw = _t(pers, (1, 1), FP, "etw")
            nc.scalar.activation(etw[:], id_f[0:1, 0:1], AF.Exp)
            bvec = []
            for kh in range(2):
                bt = _t(pers, (128, 1), FP, f"bvec{kh}")
                nc.sync.dma_start(bt[:], b_in[128 * kh : 128 * (kh + 1), 0:1])
                bvec.append(bt)

            # ---- W^T in bf16: wt[kh][k=128, d=256] = W[d, kh*128+k] ----
            wt = [_t(pers, (128, 256), BF, f"wt{kh}") for kh in range(2)]
            with (
                tc.tile_pool(name="wsb", bufs=2) as wsb,
                tc.tile_pool(name="wps", bufs=2, space="PSUM") as wps,
            ):
                for dh in range(2):
                    wl = _t(wsb, (128, 256), FP, "wl")
                    nc.sync.dma_start(wl[:], w_in[128 * dh : 128 * (dh + 1), :])
                    wc = _t(wsb, (128, 256), BF, "wc")
                    nc.vector.tensor_copy(wc[:], wl[:])
                    for kh in range(2):
                        wp = _t(wps, (128, 128), BF, "wp")
                        nc.tensor.transpose(wp[:], wc[:, 128 * kh : 128 * (kh + 1)], id_bf[:])
                        nc.vector.tensor_copy(wt[kh][:, 128 * dh : 128 * (dh + 1)], wp[:])

            # ---- persistent state ----
            # hp8b[g8][p, dh*LR + n] = H_projT[dh*128+p, g8*LR + n]  (fp8, per block
            # so it0 attention can start before the whole projection lands)
            hp8b = [_t(pers, (128, 2 * LR), F8, f"hp8b{g8}") for g8 in range(NC)]
            hp8b_3 = [t.rearrange("p (i n) -> p i n", i=2) for t in hp8b]
            # hp8_l: same layout, local rows only
            hp8_l = _t(pers, (128, 2 * LR), F8, "hp8_l")
            hp8_l3 = hp8_l.rearrange("p (i n) -> p i n", i=2)
            # qtfb[g8][kh][k, n] = Q[g8*LR + n, kh*128+k]  (bf16; it0 full proj,
            # split per key block so projection starts as soon as block 0 lands)
            qtfb = [
                [_t(pers, (128, LR), BF, f"qtf{g8}_{kh}") for kh in range(2)]
                for g8 in range(NC)
            ]
            # hT[kh][k, n] = H[n, kh*128+k]  (bf16, local rows; local proj moving)
            hT = [_t(pers, (128, LR), BF, f"hT{kh}") for kh in range(2)]
            # hpk[g]: values stationary, group g = keys [256g, 256g+256):
            #   hpk[g][p, i*256+f] = H[256g + 128i + p, f]  (fp8)
            hpk = [_t(pers, (128, 512), F8, f"hpk{g}") for g in range(NPAIR)]
            # resident local Q (f32) for the epilogue
            ql = [_t(pers, (128, D), FP, f"ql{t}") for t in range(LR // 128)]
            # additive mask (0 / -192) in S^T layout, fp8: pair tile g holds
            # chunks (2g, 2g+1): simt[g][p, i*LR + r] = mask[r_local, 256g+128i+p]
            simt = [_t(simt_pool, (128, 2 * LR), F8, f"simT{g}") for g in range(NPAIR)]
            # row-bias moving tile: partition 0 carries (coarse, residual) fp8
            # row terms; other partitions zero (stationary t2c zeroes them).
            m2 = _t(pers, (128, 2 * LR), F8, "m2")
            nc.vector.memset(m2[:], 0.0)
            m2_3 = m2.rearrange("p (i r) -> p i r", i=2)

            ot_sb = _t(pers, (128, 2 * LR), FP, "ot")
            zsb = _t(pers, (1, LR), FP, "zsb")

            # ---- collective buffers (one gather: H1 fp8 + Hp1 fp8) ----
            cc1_in = dram.tile([256, 2048], F8)
            cc1_out = dram.tile([2048, 2048], F8, addr_space="Shared")
            cc1_in_h = cc1_in.rearrange("a (b c) -> (a b) c", b=8)    # [2048, 256]
            cc1_out_h = cc1_out.rearrange("a (b c) -> (a b) c", b=8)  # [16384, 256]
            # tiny warm-up gather: absorbs the ~11us first-collective ncfw
            # latency during it0 (no data deps; runs on the CC stream)
            ccw_in = dram.tile([1, 128], BF)
            ccw_out = dram.tile([8, 128], BF, addr_space="Shared")

            # ---- upfront loads (critical-path first: proj moving, then the
            # tiles the first attention groups touch, then the long tail) ----
            for kh in range(2):
                nc.sync.dma_start(hT[kh][:], q_t[128 * kh : 128 * (kh + 1), :])
            for g8 in range(NC):
                for kh in range(2):
                    nc.sync.dma_start(
                        qtfb[g8][kh][:],
                        qt_full[128 * kh : 128 * (kh + 1), LR * g8 : LR * (g8 + 1)],
                    )
            for g in range(NPAIR):
                for i in range(2):
                    c = 2 * g + i
                    nc.sync.dma_start(
                        hpk[g][:, 256 * i : 256 * (i + 1)],
                        q8_in[256 * g + 128 * i : 256 * g + 128 * (i + 1), :],
                    )
                    nc.sync.dma_start(
                        simt[g][:, LR * i : LR * (i + 1)],
                        sim_t[128 * c : 128 * (c + 1), :],
                    )
            for t in range(LR // 128):
                nc.sync.dma_start(ql[t][:], q_loc[128 * t : 128 * (t + 1), :])

            # =================================================================
            def projection(dest_fn, n_cols, moving, on_dve=False):
                """fp8(W @ H^T + b) from bf16 moving tiles; dest_fn(dh, blk) -> AP.
                on_dve routes the bias-add/cast through the vector engine (keeps
                the scalar engine free for the attention exps)."""
                with tc.tile_pool(name="pj_mm", bufs=2, space="PSUM") as pj_mm:
                    for blk in range(n_cols // 512):
                        for dh in range(2):
                            mm = _t(pj_mm, (128, 512), FP, "hp")
                            nc.tensor.matmul(
                                mm[:], wt[0][:, 128 * dh : 128 * (dh + 1)],
                                moving[0][:, 512 * blk : 512 * (blk + 1)],
                                start=True, stop=False,
                            )
                            nc.tensor.matmul(
                                mm[:], wt[1][:, 128 * dh : 128 * (dh + 1)],
                                moving[1][:, 512 * blk : 512 * (blk + 1)],
                                start=False, stop=True,
                            )
                            if on_dve:
                                nc.vector.tensor_scalar_add(
                                    dest_fn(dh, blk), mm[:], bvec[dh][:, 0:1]
                                )
                            else:
                                nc.scalar.activation(
                                    dest_fn(dh, blk), mm[:], AF.Identity,
                                    bias=bvec[dh][:, 0:1],
                                )

            # =================================================================
            def row_bias(it):
                """m2 partition-0 <- fp8 two-term split of -(||Hp_r||^2 + MSHIFT)."""
                with (
                    tc.tile_pool(name="nm_sb", bufs=2) as nm_sb,
                    tc.tile_pool(name="nm_ps", bufs=2, space="PSUM") as nm_ps,
                ):
                    sq = _t(nm_sb, (128, 2 * LR), BF, "sq")
                    nc.vector.tensor_mul(sq[:], hp8_l[:], hp8_l[:])
                    n2_sb = _t(nm_sb, (1, LR), FP, "n2_sb")
                    for rh in range(2):
                        n2_ps = _t(nm_ps, (1, 512), FP, "n2_ps")
                        for dh in range(2):
                            nc.tensor.matmul(
                                n2_ps[:], ones_col[:],
                                sq[:, dh * LR + 512 * rh : dh * LR + 512 * (rh + 1)],
                                start=(dh == 0), stop=(dh == 1),
                            )
                        nc.vector.tensor_copy(
                            n2_sb[0:1, 512 * rh : 512 * (rh + 1)], n2_ps[:]
                        )
                    nmm = _t(nm_sb, (1, LR), FP, "nmm")
                    nc.vector.tensor_scalar(
                        nmm[:], n2_sb[:], -1.0, -MSHIFT, op0=OP.mult, op1=OP.add
                    )
                    nc.vector.tensor_copy(m2[0:1, 0:LR], nmm[:])     # coarse fp8
                    t1f = _t(nm_sb, (1, LR), FP, "t1f")
                    nc.vector.tensor_copy(t1f[:], m2[0:1, 0:LR])
                    res = _t(nm_sb, (1, LR), FP, "res")
                    nc.vector.tensor_sub(res[:], nmm[:], t1f[:])
                    nc.vector.tensor_copy(m2[0:1, LR : 2 * LR], res[:])  # residual fp8

            # =================================================================
            def attention(it):
                """PSUM: S - rowbias (fp8 DR) -> ACT exp -> DVE mask -> fp8 values.
                For it0, the redundant full projection is interleaved into
                pass 0 (sharing the sc PSUM rotation) so the PE never idles."""
                with (
                    tc.tile_pool(name="at_sb", bufs=3) as at_sb,
                    tc.tile_pool(name="at_sc", bufs=2, space="PSUM") as at_sc,
                    tc.tile_pool(name="at_o", bufs=1, space="PSUM") as at_o,
                    tc.tile_pool(name="at_z", bufs=1, space="PSUM") as at_z,
                ):
                    def proj_block(g8):
                        for blk in range(LR // 512):
                            t = _t(at_sc, (128, 1024), FP, "sc")
                            for dh in range(2):
                                for kh in range(2):
                                    nc.tensor.matmul(
                                        t[:, 512 * dh : 512 * (dh + 1)],
                                        wt[kh][:, 128 * dh : 128 * (dh + 1)],
                                        qtfb[g8][kh][:, 512 * blk : 512 * (blk + 1)],
                                        start=(kh == 0), stop=(kh == 1),
                                    )
                                nc.vector.tensor_scalar_add(
                                    hp8b[g8][:, dh * LR + 512 * blk : dh * LR + 512 * (blk + 1)],
                                    t[:, 512 * dh : 512 * (dh + 1)],
                                    bvec[dh][:, 0:1],
                                )

                    for p in range(NPASS):
                        o_ps = [_t(at_o, (128, RW), FP, f"o{dh}") for dh in range(2)]
                        z_ps = _t(at_z, (16, RW), FP, "z")
                        for g in range(NPAIR):
                            if it == 0 and p == 0 and g % 4 == 0:
                                proj_block(g // 4)
                            sc = _t(at_sc, (128, 1024), FP, "sc")
                            sgm = simt[g].rearrange("p (i r) -> p i r", i=2)
                            # scores for both halves first, then both row-bias
                            # adds: the t2c stationary loads land back-to-back
                            for i in range(2):
                                c = 2 * g + i
                                nc.tensor.matmul(
                                    sc[:, RW * i : RW * (i + 1)],
                                    hp8b_3[c // 8][:, :, 128 * (c % 8) : 128 * (c % 8 + 1)],
                                    hp8_l3[:, :, RW * p : RW * (p + 1)],
                                    start=True, stop=False, perf_mode=DR,
                                )
                            for i in range(2):
                                nc.tensor.matmul(
                                    sc[:, RW * i : RW * (i + 1)],
                                    t2c_3[:, :, :],
                                    m2_3[:, :, RW * p : RW * (p + 1)],
                                    start=False, stop=True, perf_mode=DR,
                                )
                            vexp = _t(at_sb, (128, 1024), BF, "vexp")
                            nc.scalar.activation(vexp[:], sc[:], AF.Exp)
                            vexp3 = vexp.rearrange("p (i r) -> p i r", i=2)
                            v2 = _t(at_sb, (128, 1024), F8, "v2")
                            v2m = v2.rearrange("p (i r) -> p i r", i=2)
                            nc.vector.tensor_mul(
                                v2m[:, :, :], vexp3[:, :, :],
                                sgm[:, :, RW * p : RW * (p + 1)],
                            )
                            first, last = (g == 0), (g == NPAIR - 1)
                            hpk3 = hpk[g].rearrange("p (i f) -> p i f", i=2)
                            for dh in range(2):
                                nc.tensor.matmul(
                                    o_ps[dh][:],
                                    hpk3[:, :, 128 * dh : 128 * (dh + 1)],
                                    v2m[:, :, :],
                                    start=first, stop=last, perf_mode=DR,
                                )
                            nc.tensor.matmul(
                                z_ps[:], ones2_3[:, :, :], v2m[:, :, :],
                                start=first, stop=last, perf_mode=DR,
                            )
                        for dh in range(2):
                            nc.scalar.activation(
                                ot_sb[:, 1024 * dh + RW * p : 1024 * dh + RW * (p + 1)],
                                o_ps[dh][:], AF.Copy,
                            )
                        nc.scalar.activation(
                            zsb[0:1, RW * p : RW * (p + 1)], z_ps[0:1, :], AF.Copy
                        )

            # =================================================================
            def epilogue(it, invz):
                with (
                    tc.tile_pool(name="ep_sb", bufs=3) as ep_sb,
                    tc.tile_pool(name="ep_ps", bufs=2, space="PSUM") as ep_ps,
                    tc.tile_pool(name="ep_tp", bufs=2, space="PSUM") as ep_tp,
                ):
                    zp_ps = _t(ep_tp, (128, LR // 128), FP, "zp_ps", bufs=1)
                    for t in range(LR // 128):
                        nc.tensor.transpose(
                            zp_ps[:, t : t + 1], zsb[0:1, 128 * t : 128 * (t + 1)],
                            id_f[0:1, 0:1],
                        )
                    z51 = _t(ep_sb, (128, LR // 128), FP, "z51", bufs=1)
                    nc.vector.tensor_scalar_mul(z51[:], zp_ps[:], ALPHA + BETA)
                    nc.vector.reciprocal(invz[:], z51[:])
                    for t in range(LR // 128):
                        on_ps = _t(ep_ps, (128, D), FP, "on")
                        p, sub = t // (RW // 128), t % (RW // 128)
                        for dh in range(2):
                            nc.tensor.transpose(
                                on_ps[:, 128 * dh : 128 * (dh + 1)],
                                ot_sb[:, 1024 * dh + RW * p + 128 * sub :
                                      1024 * dh + RW * p + 128 * (sub + 1)],
                                id_f[:],
                            )
                        t1 = _t(ep_sb, (128, D), FP, "t1")
                        nc.scalar.activation(
                            t1[:], on_ps[:], AF.Copy, scale=invz[:, t : t + 1]
                        )
                        qs = _t(ep_sb, (128, D), FP, "qs")
                        nc.vector.tensor_scalar_mul(
                            qs[:], ql[t][:], ALPHA / (ALPHA + BETA)
                        )
                        hnew = _t(ep_sb, (128, D), FP, "hnew")
                        nc.vector.tensor_add(hnew[:], t1[:], qs[:])
                        if it == 0:
                            # fp8 H1 for it1 values -> gather buffer
                            h8 = _t(ep_sb, (128, D), F8, "h8")
                            nc.vector.tensor_copy(h8[:], hnew[:])
                            nc.sync.dma_start(
                                cc1_in_h[1024 + 128 * t : 1024 + 128 * (t + 1), :], h8[:]
                            )
                            # bf16 H1^T for it1 projection
                            hb = _t(ep_sb, (128, D), BF, "hb")
                            nc.vector.tensor_copy(hb[:], hnew[:])
                            for kh in range(2):
                                tp = _t(ep_tp, (128, 128), BF, "tp")
                                nc.tensor.transpose(
                                    tp[:], hb[:, 128 * kh : 128 * (kh + 1)], id_bf[:]
                                )
                                nc.vector.tensor_copy(
                                    hT[kh][:, 128 * t : 128 * (t + 1)], tp[:]
                                )
                        else:
                            nc.sync.dma_start(
                                out[128 * t : 128 * (t + 1), :], hnew[:]
                            )

            # =================================================================
            for it in range(K_ITERS):
                invz = _t(pers, (128, LR // 128), FP, f"invz{it}")
                # local rows first (every score matmul needs hp8_l moving) so
                # the row-bias chain starts immediately
                projection(
                    lambda dh, blk: hp8_l[:, dh * LR + 512 * blk : dh * LR + 512 * (blk + 1)],
                    LR, hT,
                )
                row_bias(it)
                if it == 0:
                    # fire the warm-up gather during it0's attention
                    nc.sync.dma_start(ccw_in[:], id_bf_d.ap()[0:1, :])
                    nc.gpsimd.collective_compute(
                        "AllGather",
                        OP.bypass,
                        replica_groups=[list(range(NC))],
                        ins=[ccw_in.opt()],
                        outs=[ccw_out.opt()],
                    )
                else:
                    nc.sync.dma_start(cc1_in[0:128, :], hp8_l[:])
                    # bf16-bitcast views: the CCE slices by ELEMENT count, so
                    # moving the same bytes as bf16 halves the collective time
                    nc.gpsimd.collective_compute(
                        "AllGather",
                        OP.bypass,
                        replica_groups=[list(range(NC))],
                        ins=[cc1_in.bitcast(BF).opt()],
                        outs=[cc1_out.bitcast(BF).opt()],
                    )
                    # gathered projections -> hp8 block tiles
                    for g8 in range(NC):
                        for dh in range(2):
                            nc.sync.dma_start(
                                hp8b[g8][:, dh * LR : dh * LR + LR],
                                cc1_out[256 * g8 : 256 * g8 + 128,
                                        1024 * dh : 1024 * (dh + 1)],
                            )
                    # gathered H1 (fp8, flat rows) -> hpk group tiles
                    for g in range(NPAIR):
                        for i in range(2):
                            r0 = 256 * g + 128 * i
                            blk = r0 // LR
                            lr = r0 - LR * blk
                            nc.sync.dma_start(
                                hpk[g][:, 256 * i : 256 * (i + 1)],
                                cc1_out_h[2048 * blk + 1024 + lr :
                                          2048 * blk + 1024 + lr + 128, :],
                            )
                attention(it)
                epilogue(it, invz)
    nc.compile()
    return nc


def _install_ntff_hook():
    """The agent image's antenv lacks axon_hooks; synthesize it and register
    the ctypes NTFF profile hook so run_bass_kernel_spmd(trace=True) works."""
    import types

    if "antenv.axon_hooks" in sys.modules:
        return
    import antenv
    from trn_agent_boot.trn_boot import _ntff_profile_via_ctypes

    mod = types.ModuleType("antenv.axon_hooks")
    _state = {}
    mod.set_axon_ntff_profile_hook = lambda h: _state.__setitem__("h", h)
    mod.get_axon_ntff_profile_hook = lambda: _state.get("h")
    sys.modules["antenv.axon_hooks"] = mod
    antenv.axon_hooks = mod
    mod.set_axon_ntff_profile_hook(
        _ntff_profile_via_ctypes("/opt/axon/libaxon_pjrt.so")
    )


_NC_CACHE = None


def _get_nc():
    global _NC_CACHE
    if _NC_CACHE is None:
        _NC_CACHE = build()
    return _NC_CACHE


def kernel(Q, sim_mat, W, b, _trace=False, _trace_kwargs=None):
    Q = np.ascontiguousarray(np.asarray(Q, dtype=np.float32))
    sim_mat = np.ascontiguousarray(np.asarray(sim_mat, dtype=np.float32))
    W = np.ascontiguousarray(np.asarray(W, dtype=np.float32))
    b = np.ascontiguousarray(np.asarray(b, dtype=np.float32)).reshape(D, 1)

    s8m = sim_mat.astype(F8NP)                    # 0/1 multiplicative mask
    q8 = np.ascontiguousarray(Q.astype(F8NP))
    qtf = np.ascontiguousarray(Q.T.astype(BFNP))

    in_maps = []
    for g in range(NC):
        sl = slice(g * LR, (g + 1) * LR)
        in_maps.append(
            {
                "sim_t": np.ascontiguousarray(s8m[sl].T),
                "qt_full": qtf,
                "q_t": np.ascontiguousarray(Q[sl].T.astype(BFNP)),
                "q8": q8,
                "q_loc": np.ascontiguousarray(Q[sl]),
                "w": W,
                "b": b,
            }
        )
    nc = _get_nc()
    kw = {}
    if _trace:
        _install_ntff_hook()
        kw["trace"] = True
        kw.update(_trace_kwargs or {})
    res = run_bass_kernel_spmd(nc, in_maps, core_ids=list(range(NC)), **kw)
    outp = np.concatenate(
        [np.asarray(res.results[g]["out"]).reshape(LR, D) for g in range(NC)], axis=0
    ).astype(np.float32)
    if _trace:
        return outp, res
    return outp


if __name__ == "__main__":
    nc = build()
    print("build+compile OK")
